# revision 22
# baseline (speedup 1.0000x reference)
"""Trainium2 Bass kernel for nn_Convolution (e3nn-style GNN message passing).

Strategy (8 NeuronCores, SPMD, no collectives; ~790us HW exec):
- The 391 global 128-node dst blocks are LPT-assigned to cores (50 slots each)
  to balance edge counts. Each core processes its blocks largest-first; the
  shared program structure ngb[k] = ceil(max over cores of the k-th largest
  block count / 128) groups of 128 edges per position (program is specialized
  to the input's block-count histogram; compiled once and cached). The host
  unpermutes position-ordered outputs back to node order.
- Dummy (padding) edges gather a zero table row and carry an all-zero dst
  one-hot column, so they contribute nothing.
- Gather source features (bf16 table, 256B rows) with gpsimd dma_gather,
  1024 idx per call (the HW cap; 2048 faults). This is the bottleneck:
  ~7.6ns/descriptor of serial Pool-engine time.
- Radial MLP layer 1 on PE (bf16, tile_position row-packed K=8 matmuls),
  layer 2 per-group with h as the stationary operand; w lands [edge, 256] in
  PSUM, then is copied to bf16 SBUF on the scalar engine.
- TP products on DVE, all bf16 with stride-1 innermost APs (outer-dim
  broadcasts only — inner-strided/broadcast APs defeat DVE vectorization).
  Layouts:
    scat[0:128]   = (w8, u'16) : [s1*s2 | v1.v2] x w01
    scat[128:320] = (i3, w8, u8): s1v2(i,u) * w2(w,u)
    scat[320:512] = (i3, w8, u8): v1s2(i,u) * w3(w,u)
  The contraction over u is DEFERRED into the scatter matmul.
- Scatter: one-hot(dst) columns are precomputed on the HOST and streamed in
  (no on-device is_equal); one matmul per group accumulates in PSUM per
  block position; a reduce over (path, u) finishes the 32 outputs.
- The window loop is software-pipelined: produce(w) = streams/gather/MLPs,
  consume(w-1) = TP products + scatter, so the in-order engine queues always
  hold ready work (PE/ACT/DVE overlap across windows).
"""

import math
import os
import ml_dtypes
import numpy as np

BF16 = ml_dtypes.bfloat16

import concourse.bass as bass
import concourse.bacc as bacc
import concourse.mybir as mybir
from concourse.tile import TileContext
from concourse.bass_utils import run_bass_kernel_spmd

# ---------------- problem constants (hardcoded per spec) ----------------
N_NODES, N_EDGES, NUM_BASIS, HIDDEN = 50000, 800000, 8, 256
MUL = 8
INV_SQRT3 = float(1.0 / np.sqrt(3.0))
A_SCALAR = float(np.sqrt(1.0 / 128.0))
A_VECTOR = float(np.sqrt(3.0 / 128.0))
SQRT2 = float(np.sqrt(2.0))
DEG_SCALE = float(1.0 / np.sqrt(N_EDGES / N_NODES))

NCORES = 8
P = 128
NODES_PER_CORE = 6400          # 50 blocks of 128; 8*6400 = 51200 >= 50000
NB = 50                        # node blocks per core
# table: rows 1..50000 = nodes 0..49999; row 50001 = zeros (dummy target).
# gather base = row 32768, int16 idx = node - 32767 in [-32767, 17232];
# dummy idx = +17233 (non-negative so it never hits the trailing-negative
# trim). Each gather's last (trim-order) index is forced >= 0 by an in-block
# edge swap on the host.
TBL_ROWS = 50004
GBASE = 32768
DUMMY_IDX = 50001 - GBASE

_TRACE_SIM = bool(int(os.environ.get('K_TRACE_SIM', '0')))

_PROG_CACHE = {}


# ---------------- device program ----------------
def _build_program(ngb):
    # ngb[k] = number of 128-edge groups at block position k (same for all
    # cores: cross-core max of descending-sorted per-core block counts).
    layout = []                      # per group: (position, first, last)
    for k, n in enumerate(ngb):
        for gib in range(n):
            layout.append((k, gib == 0, gib == n - 1))
    GROUPS = len(layout)
    assert GROUPS % 4 == 0
    WINDOWS = GROUPS // 4            # 4 groups (512 edges) per window
    NSUPER = (WINDOWS + 1) // 2      # one gather per 2 windows (1024 idx)
    NJ = (WINDOWS + 3) // 4          # es_w4 column blocks
    ES_CHUNK_J = 3                   # es col-blocks per streamed chunk
    CH_W = 4                         # shx/ohs windows per streamed chunk

    nc = bacc.Bacc(num_devices=NCORES, num_swdge_queues=4)
    f32, i16 = mybir.dt.float32, mybir.dt.int16
    bf16 = mybir.dt.bfloat16

    tbl = nc.dram_tensor("tbl", [TBL_ROWS, 128], bf16, kind="ExternalInput")
    idx_g = nc.dram_tensor("idx_g", [P, GROUPS * 8], i16, kind="ExternalInput")
    es4 = nc.dram_tensor("es4", [P, NJ * 512], bf16, kind="ExternalInput")
    shx = nc.dram_tensor("shx", [P, GROUPS * 72], bf16, kind="ExternalInput")
    ohs = nc.dram_tensor("ohs", [P, GROUPS * 128], bf16, kind="ExternalInput")
    w1t = nc.dram_tensor("w1t", [P, 256], bf16, kind="ExternalInput")
    w2t = nc.dram_tensor("w2t", [P, 512], bf16, kind="ExternalInput")
    nodeout = nc.dram_tensor("nodeout", [NODES_PER_CORE, 32], f32, kind="ExternalOutput")

    AX = mybir.AxisListType.X
    AXY = mybir.AxisListType.XY
    ADD = mybir.AluOpType.add
    MUL_ = mybir.AluOpType.mult
    RELU = mybir.ActivationFunctionType.Relu

    with TileContext(nc, trace_sim=_TRACE_SIM) as tc:
        with tc.tile_pool(name="const", bufs=1) as cpool, \
             tc.tile_pool(name="stream", bufs=3) as spool, \
             tc.tile_pool(name="xcp", bufs=4) as xcpool, \
             tc.tile_pool(name="work", bufs=2) as wpool, \
             tc.tile_pool(name="psum", bufs=2, space="PSUM") as pp, \
             tc.tile_pool(name="psum1", bufs=1, space="PSUM") as pp1:

            # constants resident in SBUF
            ig_sb = cpool.tile([P, GROUPS * 8], i16, name="ig")
            ig_head = min(16 * 64, GROUPS * 8)
            nc.sync.dma_start(ig_sb[:, :ig_head], idx_g[:, :ig_head])
            if GROUPS * 8 > ig_head:
                nc.sync.dma_start(ig_sb[:, ig_head:], idx_g[:, ig_head:])
            w1_sb = cpool.tile([P, 256], bf16, name="w1")
            nc.sync.dma_start(w1_sb[:], w1t[:])
            w2_sb = cpool.tile([P, 2, 256], bf16, name="w2")
            nc.sync.dma_start(w2_sb[:], w2t[:].rearrange("p (h n) -> p h n", h=2))

            # software-pipelined: produce(w) = streams/gather/MLP; consume(w)
            # = TP products + scatter, one window behind, so each engine's
            # in-order queue always has ready work from the other window.
            ctx = {}
            xcs = {}
            state = dict(acc_ps=None, es_sb=None, shx_sb=None, ohs_sb=None)

            def issue_gather(s):
                # gathers source rows for windows 2s, 2s+1
                if s >= NSUPER:
                    return
                n_idx = min(1024, (WINDOWS - 2 * s) * 512)
                xcs[s] = xcpool.tile([P, 8, 128], bf16, tag="xc", name="x_c")
                nc.gpsimd.dma_gather(
                    out_ap=xcs[s][:, : n_idx // 128, :],
                    in_ap=tbl[GBASE:, :],
                    idxs_ap=ig_sb[:, s * 64 : s * 64 + n_idx // 16],
                    num_idxs=n_idx, num_idxs_reg=n_idx, elem_size=128,
                    queue_num=s % 4,
                )

            def produce(w):
                c = w % 4
                j = w // 4
                g0 = 4 * w

                if w % (4 * ES_CHUNK_J) == 0:
                    jw = min(ES_CHUNK_J, NJ - j)
                    state["es_sb"] = spool.tile(
                        [P, ES_CHUNK_J * 512], bf16, tag="es", name="es_sb"
                    )
                    nc.sync.dma_start(
                        state["es_sb"][:, : jw * 512],
                        es4[:, j * 512 : (j + jw) * 512],
                    )
                jj = j % ES_CHUNK_J
                es_sb = state["es_sb"]

                if w % CH_W == 0:
                    cw = min(CH_W, WINDOWS - w)
                    state["shx_sb"] = spool.tile([P, CH_W, 4, 72], bf16, tag="shx", name="shx_sb")
                    nc.sync.dma_start(
                        state["shx_sb"][:, :cw, :, :].rearrange("p a b c -> p (a b c)"),
                        shx[:, g0 * 72 : (g0 + 4 * cw) * 72],
                    )
                    state["ohs_sb"] = spool.tile([P, CH_W, 4, 128], bf16, tag="ohs", name="ohs_sb")
                    nc.sync.dma_start(
                        state["ohs_sb"][:, :cw, :, :].rearrange("p a b c -> p (a b c)"),
                        ohs[:, g0 * 128 : (g0 + 4 * cw) * 128],
                    )

                # gather runs 2 supers (4 windows) ahead of this window
                if w % 2 == 0:
                    issue_gather(w // 2 + 2)

                # MLP1: h[comp, edge] for 512 edges, two 128-comp halves
                h_ps = pp1.tile([P, 2, 512], f32, space="PSUM", tag="hps")
                for half in range(2):
                    nc.tensor.matmul(
                        h_ps[:, half, :],
                        lhsT=w1_sb[32 * c : 32 * c + 8, half * 128 : half * 128 + 128],
                        rhs=es_sb[32 * c : 32 * c + 8, jj * 512 : jj * 512 + 512],
                        start=True, stop=True,
                        tile_position=(32 * c, 0),
                    )
                h_sb = wpool.tile([P, 2, 512], bf16, tag="hsb")
                nc.scalar.activation(out=h_sb[:], in_=h_ps[:], func=RELU)

                # MLP2 per group: w[edge, 256] in PSUM -> bf16 SBUF copy
                w_ps = pp.tile([P, 4, 256], f32, space="PSUM", tag="wps")
                for gg in range(4):
                    for half in range(2):
                        nc.tensor.matmul(
                            w_ps[:, gg, :],
                            lhsT=h_sb[:, half, gg * 128 : gg * 128 + 128],
                            rhs=w2_sb[:, half, :],
                            start=(half == 0), stop=(half == 1),
                        )
                w_sb = wpool.tile([P, 4, 256], bf16, tag="wsb")
                nc.scalar.copy(w_sb[:], w_ps[:])

                ctx[w] = (xcs[w // 2], state["shx_sb"], state["ohs_sb"], w_sb)

            def consume(w):
                g0 = 4 * w
                lw = w % CH_W
                x_c, shx_sb, ohs_sb, w_sb = ctx.pop(w)
                if w % 2 == 1:
                    xcs.pop(w // 2, None)
                xs = x_c[:, 4 * (w % 2) : 4 * (w % 2) + 4, :]   # [P, 4, 128]

                # TP products (batched over the 4 groups, all bf16)
                shw = shx_sb[:, lw, :, :]             # [P, 4, 72]
                s2r = shw[:, :, 0:24]                 # s2 repeated 24x
                v2A = shw[:, :, 24:48]                # v2 as (u8, i3)
                v2B = shw[:, :, 48:72]                # v2 as (i3, u8)

                s1s2 = wpool.tile([P, 4, 8], bf16, tag="s1s2")
                nc.vector.tensor_tensor(
                    out=s1s2[:], in0=xs[:, :, 0:8], in1=s2r[:, :, 0:8], op=MUL_
                )
                pb = wpool.tile([P, 4, 24], bf16, tag="pb")
                nc.vector.tensor_tensor(
                    out=pb[:], in0=xs[:, :, 8:32], in1=v2A, op=MUL_
                )
                bb = wpool.tile([P, 4, 8], bf16, tag="bb")
                with nc.allow_low_precision("bf16 dot over i=3"):
                    nc.vector.tensor_reduce(
                        out=bb[:],
                        in_=pb[:].rearrange("p g (u i) -> p g u i", u=8),
                        axis=AX, op=ADD,
                    )
                s1v2 = wpool.tile([P, 4, 24], bf16, tag="s1v2")
                nc.vector.tensor_tensor(
                    out=s1v2[:].rearrange("p g (i u) -> p g i u", i=3),
                    in0=xs[:, :, 0:8].unsqueeze(2).to_broadcast([P, 4, 3, 8]),
                    in1=v2B.rearrange("p g (i u) -> p g i u", i=3),
                    op=MUL_,
                )
                v1s2 = wpool.tile([P, 4, 24], bf16, tag="v1s2")
                nc.vector.tensor_tensor(
                    out=v1s2[:].rearrange("p g (i u) -> p g i u", i=3),
                    in0=xs[:, :, 8:32].rearrange("p g (u i) -> p g i u", u=8),
                    in1=s2r[:].rearrange("p g (i u) -> p g i u", i=3),
                    op=MUL_,
                )

                scat = wpool.tile([P, 4, 512], bf16, tag="scat")
                # path01: scat[(w8, u'16)] = [s1s2 | bb] * w01(w, u')
                w01 = w_sb[:, :, 0:128].rearrange("p g (w u) -> p g w u", w=8)
                sc01 = scat[:, :, 0:128].rearrange("p g (w u) -> p g w u", w=8)
                nc.vector.tensor_tensor(
                    out=sc01[:, :, :, 0:8],
                    in0=s1s2[:].unsqueeze(2).to_broadcast([P, 4, 8, 8]),
                    in1=w01[:, :, :, 0:8], op=MUL_,
                )
                nc.vector.tensor_tensor(
                    out=sc01[:, :, :, 8:16],
                    in0=bb[:].unsqueeze(2).to_broadcast([P, 4, 8, 8]),
                    in1=w01[:, :, :, 8:16], op=MUL_,
                )
                # path2: scat[(i3, w8, u8)] = s1v2(i,u) * w2(w,u)
                nc.vector.tensor_tensor(
                    out=scat[:, :, 128:320].rearrange(
                        "p g (i w u) -> p g i w u", i=3, w=8
                    ),
                    in0=s1v2[:].rearrange("p g (i u) -> p g i u", i=3)
                        .unsqueeze(3).to_broadcast([P, 4, 3, 8, 8]),
                    in1=w_sb[:, :, 128:192]
                        .rearrange("p g (w u) -> p g w u", w=8)
                        .unsqueeze(2).to_broadcast([P, 4, 3, 8, 8]),
                    op=MUL_,
                )
                # path3: scat[(i3, w8, u8)] = v1s2(i,u) * w3(w,u)
                nc.vector.tensor_tensor(
                    out=scat[:, :, 320:512].rearrange(
                        "p g (i w u) -> p g i w u", i=3, w=8
                    ),
                    in0=v1s2[:].rearrange("p g (i u) -> p g i u", i=3)
                        .unsqueeze(3).to_broadcast([P, 4, 3, 8, 8]),
                    in1=w_sb[:, :, 192:256]
                        .rearrange("p g (w u) -> p g w u", w=8)
                        .unsqueeze(2).to_broadcast([P, 4, 3, 8, 8]),
                    op=MUL_,
                )

                # per group: streamed one-hot + scatter matmul into block acc
                for gg in range(4):
                    g = g0 + gg
                    b, first, last = layout[g]
                    if first:
                        state["acc_ps"] = pp.tile(
                            [P, 512], f32, space="PSUM", tag="acc", name="acc_ps"
                        )
                    acc_ps = state["acc_ps"]
                    nc.tensor.matmul(
                        acc_ps[:],
                        lhsT=ohs_sb[:, lw, gg, :], rhs=scat[:, gg, :],
                        start=first, stop=last,
                    )
                    if last:
                        stage = wpool.tile([P, 32], f32, tag="stage")
                        nc.vector.tensor_reduce(
                            out=stage[:, 0:8],
                            in_=acc_ps[:, 0:128].rearrange(
                                "p (w u) -> p w u", w=8
                            ),
                            axis=AX, op=ADD,
                        )
                        nc.vector.tensor_reduce(
                            out=stage[:, 8:32],
                            in_=acc_ps[:, 128:512].rearrange(
                                "p (t i w u) -> p w i t u", t=2, i=3, w=8
                            ),
                            axis=AXY, op=ADD,
                        )
                        nc.sync.dma_start(
                            nodeout[128 * b : 128 * b + 128, :], stage[:]
                        )

            issue_gather(0)
            issue_gather(1)
            for w in range(WINDOWS + 1):
                if w < WINDOWS:
                    produce(w)
                if w >= 1:
                    consume(w - 1)
    nc.compile()
    return nc


# ---------------- host-side prep ----------------
def _prep(node_features, edge_src, edge_dst, edge_sh, edge_scalars, fc_w1, fc_w2,
          ngb, assign):
    ngb = np.asarray(ngb, np.int64)
    GROUPS = int(ngb.sum())
    EPAD = GROUPS * 128
    WINDOWS = GROUPS // 4
    NSUPER = (WINDOWS + 1) // 2
    NJ = (WINDOWS + 3) // 4
    gstart = np.zeros(NB, np.int64)
    gstart[1:] = np.cumsum(ngb)[:-1]
    k_of_group = np.repeat(np.arange(NB), ngb)

    # fold all scalar coefficients into the weights
    w1s = (fc_w1 * (1.0 / math.sqrt(NUM_BASIS))).astype(np.float32)     # [8, 256]
    w2 = (fc_w2 * (SQRT2 / math.sqrt(HIDDEN))).astype(np.float64)       # [256, 256]
    w2 = w2.reshape(HIDDEN, 4, MUL, MUL)
    coef = np.array(
        [A_SCALAR, A_SCALAR * INV_SQRT3, A_VECTOR * INV_SQRT3, A_VECTOR * INV_SQRT3]
    ) * DEG_SCALE
    w2 = w2 * coef[None, :, None, None]
    # device col order: [p01 (w8,u'16) | p2 (w8,u8) | p3 (w8,u8)], value a[u,w]
    p01 = np.concatenate([w2[:, 0], w2[:, 1]], axis=1)      # [H, u'16, w8]
    w2dev = np.concatenate(
        [
            p01.transpose(0, 2, 1).reshape(HIDDEN, 128),
            w2[:, 2].transpose(0, 2, 1).reshape(HIDDEN, 64),
            w2[:, 3].transpose(0, 2, 1).reshape(HIDDEN, 64),
        ],
        axis=1,
    ).astype(np.float32)                                                # [256, 256]

    w1t = np.zeros((P, 256), BF16)
    for c in range(4):
        w1t[32 * c : 32 * c + 8] = w1s.astype(BF16)
    w2t = np.zeros((P, 512), BF16)
    w2t[:, 0:256] = w2dev[0:128].astype(BF16)
    w2t[:, 256:512] = w2dev[128:256].astype(BF16)

    tbl = np.zeros((TBL_ROWS, 128), BF16)
    tbl[1 : N_NODES + 1, 0:32] = node_features.astype(BF16)

    src_all = np.asarray(edge_src).astype(np.int64)
    dst_all = np.asarray(edge_dst).astype(np.int64)
    es_all = np.asarray(edge_scalars).astype(np.float32)
    sh_all = np.asarray(edge_sh).astype(np.float32)
    core_of = assign[dst_all >> 7]

    # per-super gather boundaries (1024 idx per super; last may be shorter)
    ends = [min((si + 1) * 1024, EPAD) - 1 for si in range(NSUPER)]
    end_set = set(ends)

    in_maps = []
    bops = []
    for cid in range(NCORES):
        myblocks = np.nonzero(assign == cid)[0]          # global block ids
        local_of = np.full(NBLK, -1, np.int64)
        local_of[myblocks] = np.arange(len(myblocks))
        sel = np.nonzero(core_of == cid)[0]
        d = dst_all[sel]
        blk = local_of[d >> 7]                           # local block id
        order = np.argsort(blk, kind="stable")
        sel = sel[order]
        d = d[order]
        blk = blk[order]
        cnt = np.bincount(blk, minlength=NB)
        block_of_pos = np.argsort(-cnt, kind="stable")   # position k -> local blk
        pos_of_block = np.empty(NB, np.int64)
        pos_of_block[block_of_pos] = np.arange(NB)
        assert np.all(cnt[block_of_pos] <= ngb * 128), cid
        # position k -> GLOBAL block id (or -1 for pad slots)
        gb_of_pos = np.full(NB, -1, np.int64)
        valid = block_of_pos < len(myblocks)
        gb_of_pos[valid] = myblocks[block_of_pos[valid]]
        bops.append(gb_of_pos)
        start = np.zeros(NB, np.int64)
        start[1:] = np.cumsum(cnt)[:-1]
        rank = np.arange(len(sel)) - start[blk]
        slot = gstart[pos_of_block[blk]] * 128 + rank

        srcv = np.full(EPAD, -1, np.int64)
        srcv[slot] = src_all[sel]
        shv = np.zeros((EPAD, 4), np.float32)
        shv[slot] = sh_all[sel]
        esv = np.zeros((EPAD, 8), np.float32)
        esv[slot] = es_all[sel]
        dlv = np.full(EPAD, -1.0, np.float32)
        dlv[slot] = (d & 127).astype(np.float32)

        # gather indices: row = node+1, idx = row - GBASE; dummy -> DUMMY_IDX
        idxv = np.where(srcv >= 0, srcv + 1 - GBASE, DUMMY_IDX).astype(np.int64)
        # force the trim-order-last index of each gather call to be >= 0 by
        # swapping that edge with a non-negative-idx edge of the SAME node
        # block (any within-block permutation is valid).
        for jl in ends:
            if idxv[jl] >= 0:
                continue
            k0 = k_of_group[jl // 128]
            lo = int(gstart[k0]) * 128
            hi = lo + int(ngb[k0]) * 128
            cand = np.nonzero(idxv[lo:hi] >= 0)[0]
            cand = [lo + q for q in cand if (lo + q) not in end_set]
            assert cand, "no swap candidate in block"
            q = cand[0]
            for arr in (idxv, srcv, dlv):
                arr[jl], arr[q] = arr[q], arr[jl]
            for arr in (shv, esv):
                tmpq = arr[q].copy()
                arr[q] = arr[jl]
                arr[jl] = tmpq
        idx_g = np.tile(
            idxv.reshape(-1, 16).T.astype(np.int16), (8, 1)
        )  # [128, EPAD/16], wrap is uniform

        # es4: window w at rows 32*(w%4)+b, cols [ (w//4)*512, +512 )
        es4 = np.zeros((P, NJ * 512), BF16)
        esw = esv.reshape(WINDOWS, 512, 8)
        for c in range(4):
            wsel = np.arange(c, WINDOWS, 4)       # these windows use strip c
            nw = len(wsel)
            es4[32 * c : 32 * c + 8, : nw * 512] = (
                esw[wsel].transpose(2, 0, 1).reshape(8, nw * 512).astype(BF16)
            )

        # shx: [P, G, 72] = [s2 x24 | v2 as (u8,i3) | v2 as (i3,u8)]
        s2g = shv[:, 0].reshape(GROUPS, P).T                  # [P, G]
        v2g = shv[:, 1:4].reshape(GROUPS, P, 3).transpose(1, 0, 2)  # [P, G, 3]
        shx = np.empty((P, GROUPS, 72), BF16)
        shx[:, :, 0:24] = s2g[:, :, None]
        shx[:, :, 24:48] = np.tile(v2g, (1, 1, 8))
        shx[:, :, 48:72] = np.repeat(v2g, 8, axis=2)

        # ohs: [P, G, 128] one-hot of local dst (dummies: all-zero row)
        ohs = (
            dlv.reshape(GROUPS, P).T[:, :, None]
            == np.arange(128, dtype=np.float32)
        ).astype(BF16)

        in_maps.append(
            dict(
                tbl=tbl, idx_g=np.ascontiguousarray(idx_g),
                es4=np.ascontiguousarray(es4),
                shx=np.ascontiguousarray(shx.reshape(P, GROUPS * 72)),
                ohs=np.ascontiguousarray(ohs.reshape(P, GROUPS * 128)),
                w1t=w1t, w2t=w2t,
            )
        )
    return in_maps, bops


NBLK = (N_NODES + 127) // 128                       # 391 global 128-node blocks


def _compute_struct(edge_dst):
    """LPT-assign global blocks to cores (balance edge loads), then derive the
    shared group structure: ngb[k] = ceil(max over cores of the k-th largest
    per-core block count / 128)."""
    dst_all = np.asarray(edge_dst).astype(np.int64)
    cnt = np.bincount(dst_all >> 7, minlength=NBLK)
    order = np.argsort(-cnt, kind="stable")
    loads = np.zeros(NCORES, np.int64)
    nblocks = np.zeros(NCORES, np.int64)
    assign = np.zeros(NBLK, np.int64)
    for b in order:
        cands = [i for i in range(NCORES) if nblocks[i] < NB]
        c = min(cands, key=lambda i: loads[i])
        assign[b] = c
        loads[c] += cnt[b]
        nblocks[c] += 1
    percore = np.zeros((NCORES, NB), np.int64)
    for c in range(NCORES):
        sel = cnt[assign == c]
        percore[c, : len(sel)] = sel
    s = -np.sort(-percore, axis=1)                   # descending per core
    ngb = np.maximum(np.ceil(s.max(axis=0) / 128.0).astype(np.int64), 1)
    ngb[-1] += (-int(ngb.sum())) % 4                 # pad GROUPS to %4
    return tuple(int(v) for v in ngb), assign


def kernel(node_features, edge_src, edge_dst, edge_sh, edge_scalars, fc_w1, fc_w2):
    node_features = np.asarray(node_features, dtype=np.float32)
    edge_sh = np.asarray(edge_sh, dtype=np.float32)
    edge_scalars = np.asarray(edge_scalars, dtype=np.float32)
    fc_w1 = np.asarray(fc_w1, dtype=np.float32)
    fc_w2 = np.asarray(fc_w2, dtype=np.float32)

    ngb, assign = _compute_struct(edge_dst)
    if ngb not in _PROG_CACHE:
        _PROG_CACHE[ngb] = _build_program(ngb)
    nc = _PROG_CACHE[ngb]

    in_maps, bops = _prep(
        node_features, edge_src, edge_dst, edge_sh, edge_scalars, fc_w1, fc_w2,
        ngb, assign,
    )
    res = run_bass_kernel_spmd(nc, in_maps, core_ids=list(range(NCORES)))
    return _assemble([res.results[c]["nodeout"] for c in range(NCORES)], bops)


def _assemble(nodeouts, bops):
    out = np.zeros((NBLK * 128, 32), np.float32)
    for c in range(NCORES):
        r = nodeouts[c]
        for k in range(NB):
            gb = bops[c][k]
            if gb >= 0:
                out[128 * gb : 128 * gb + 128] = r[128 * k : 128 * k + 128]
    return out[:N_NODES].astype(np.float32)


# revision 23
# speedup vs baseline: 1.1979x; 1.1979x over previous
"""Trainium2 Bass kernel for nn_Convolution (e3nn-style GNN message passing).

Strategy (8 NeuronCores, SPMD, no collectives; ~790us HW exec):
- The 391 global 128-node dst blocks are LPT-assigned to cores (50 slots each)
  to balance edge counts. Each core processes its blocks largest-first; the
  shared program structure ngb[k] = ceil(max over cores of the k-th largest
  block count / 128) groups of 128 edges per position (program is specialized
  to the input's block-count histogram; compiled once and cached). The host
  unpermutes position-ordered outputs back to node order.
- Dummy (padding) edges gather a zero table row and carry an all-zero dst
  one-hot column, so they contribute nothing.
- Gather source features (bf16 table, 256B rows) with gpsimd dma_gather,
  1024 idx per call (the HW cap; 2048 faults). This is the bottleneck:
  ~7.6ns/descriptor of serial Pool-engine time.
- Radial MLP layer 1 on PE (bf16, tile_position row-packed K=8 matmuls),
  layer 2 per-group with h as the stationary operand; w lands [edge, 256] in
  PSUM, then is copied to bf16 SBUF on the scalar engine.
- TP products on DVE, all bf16 with stride-1 innermost APs (outer-dim
  broadcasts only — inner-strided/broadcast APs defeat DVE vectorization).
  Layouts:
    scat[0:128]   = (w8, u'16) : [s1*s2 | v1.v2] x w01
    scat[128:320] = (i3, w8, u8): s1v2(i,u) * w2(w,u)
    scat[320:512] = (i3, w8, u8): v1s2(i,u) * w3(w,u)
  The contraction over u is DEFERRED into the scatter matmul.
- Scatter: one-hot(dst) columns are precomputed on the HOST and streamed in
  (no on-device is_equal); one matmul per group accumulates in PSUM per
  block position; a reduce over (path, u) finishes the 32 outputs.
- The window loop is software-pipelined: produce(w) = streams/gather/MLPs,
  consume(w-1) = TP products + scatter, so the in-order engine queues always
  hold ready work (PE/ACT/DVE overlap across windows).
"""

import math
import os
import ml_dtypes
import numpy as np

BF16 = ml_dtypes.bfloat16

import concourse.bass as bass
import concourse.bacc as bacc
import concourse.mybir as mybir
from concourse.tile import TileContext
from concourse.bass_utils import run_bass_kernel_spmd

# ---------------- problem constants (hardcoded per spec) ----------------
N_NODES, N_EDGES, NUM_BASIS, HIDDEN = 50000, 800000, 8, 256
MUL = 8
INV_SQRT3 = float(1.0 / np.sqrt(3.0))
A_SCALAR = float(np.sqrt(1.0 / 128.0))
A_VECTOR = float(np.sqrt(3.0 / 128.0))
SQRT2 = float(np.sqrt(2.0))
DEG_SCALE = float(1.0 / np.sqrt(N_EDGES / N_NODES))

NCORES = 8
P = 128
NODES_PER_CORE = 6400          # 50 blocks of 128; 8*6400 = 51200 >= 50000
NB = 50                        # node blocks per core
# table: rows 1..50000 = nodes 0..49999; row 50001 = zeros (dummy target).
# gather base = row 32768, int16 idx = node - 32767 in [-32767, 17232];
# dummy idx = +17233 (non-negative so it never hits the trailing-negative
# trim). Each gather's last (trim-order) index is forced >= 0 by an in-block
# edge swap on the host.
TBL_ROWS = 50004
GBASE = 32768
DUMMY_IDX = 50001 - GBASE

_TRACE_SIM = bool(int(os.environ.get('K_TRACE_SIM', '0')))

_PROG_CACHE = {}


# ---------------- device program ----------------
def _build_program(ngb):
    # ngb[k] = number of 128-edge groups at block position k (same for all
    # cores: cross-core max of descending-sorted per-core block counts).
    layout = []                      # per group: (position, first, last)
    for k, n in enumerate(ngb):
        for gib in range(n):
            layout.append((k, gib == 0, gib == n - 1))
    GROUPS = len(layout)
    assert GROUPS % 4 == 0
    WINDOWS = GROUPS // 4            # 4 groups (512 edges) per window
    NSUPER = (WINDOWS + 1) // 2      # one gather per 2 windows (1024 idx)
    NJ = (WINDOWS + 3) // 4          # es_w4 column blocks
    ES_CHUNK_J = 3                   # es col-blocks per streamed chunk
    CH_W = 4                         # shx/ohs windows per streamed chunk

    nc = bacc.Bacc(num_devices=NCORES, num_swdge_queues=4)
    f32, i16 = mybir.dt.float32, mybir.dt.int16
    bf16 = mybir.dt.bfloat16

    tbl = nc.dram_tensor("tbl", [TBL_ROWS, 128], bf16, kind="ExternalInput")
    idx_g = nc.dram_tensor("idx_g", [P, GROUPS * 8], i16, kind="ExternalInput")
    es4 = nc.dram_tensor("es4", [P, NJ * 512], bf16, kind="ExternalInput")
    shx = nc.dram_tensor("shx", [P, GROUPS * 72], bf16, kind="ExternalInput")
    ohs = nc.dram_tensor("ohs", [P, GROUPS * 128], bf16, kind="ExternalInput")
    w1t = nc.dram_tensor("w1t", [P, 256], bf16, kind="ExternalInput")
    w2t = nc.dram_tensor("w2t", [P, 512], bf16, kind="ExternalInput")
    nodeout = nc.dram_tensor("nodeout", [NODES_PER_CORE, 32], f32, kind="ExternalOutput")

    AX = mybir.AxisListType.X
    AXY = mybir.AxisListType.XY
    ADD = mybir.AluOpType.add
    MUL_ = mybir.AluOpType.mult
    RELU = mybir.ActivationFunctionType.Relu

    with TileContext(nc, trace_sim=_TRACE_SIM) as tc:
        with tc.tile_pool(name="const", bufs=1) as cpool, \
             tc.tile_pool(name="stream", bufs=3) as spool, \
             tc.tile_pool(name="xcp", bufs=4) as xcpool, \
             tc.tile_pool(name="work", bufs=2) as wpool, \
             tc.tile_pool(name="psum", bufs=2, space="PSUM") as pp, \
             tc.tile_pool(name="psum1", bufs=1, space="PSUM") as pp1:

            # constants resident in SBUF
            ig_sb = cpool.tile([P, GROUPS * 8], i16, name="ig")
            ig_head = min(16 * 64, GROUPS * 8)
            nc.sync.dma_start(ig_sb[:, :ig_head], idx_g[:, :ig_head])
            if GROUPS * 8 > ig_head:
                nc.sync.dma_start(ig_sb[:, ig_head:], idx_g[:, ig_head:])
            w1_sb = cpool.tile([P, 256], bf16, name="w1")
            nc.sync.dma_start(w1_sb[:], w1t[:])
            w2_sb = cpool.tile([P, 2, 256], bf16, name="w2")
            nc.sync.dma_start(w2_sb[:], w2t[:].rearrange("p (h n) -> p h n", h=2))

            # software-pipelined: produce(w) = streams/gather/MLP; consume(w)
            # = TP products + scatter, one window behind, so each engine's
            # in-order queue always has ready work from the other window.
            ctx = {}
            xcs = {}
            state = dict(acc_ps=None, es_sb=None, shx_sb=None, ohs_sb=None)

            def issue_gather(s):
                # gathers source rows for windows 2s, 2s+1
                if s >= NSUPER:
                    return
                n_idx = min(1024, (WINDOWS - 2 * s) * 512)
                xcs[s] = xcpool.tile([P, 8, 128], bf16, tag="xc", name="x_c")
                nc.gpsimd.dma_gather(
                    out_ap=xcs[s][:, : n_idx // 128, :],
                    in_ap=tbl[GBASE:, :],
                    idxs_ap=ig_sb[:, s * 64 : s * 64 + n_idx // 16],
                    num_idxs=n_idx, num_idxs_reg=n_idx, elem_size=128,
                    queue_num=s % 4,
                )

            def produce(w):
                c = w % 4
                j = w // 4
                g0 = 4 * w

                if w % (4 * ES_CHUNK_J) == 0:
                    jw = min(ES_CHUNK_J, NJ - j)
                    state["es_sb"] = spool.tile(
                        [P, ES_CHUNK_J * 512], bf16, tag="es", name="es_sb"
                    )
                    nc.sync.dma_start(
                        state["es_sb"][:, : jw * 512],
                        es4[:, j * 512 : (j + jw) * 512],
                    )
                jj = j % ES_CHUNK_J
                es_sb = state["es_sb"]

                if w % CH_W == 0:
                    cw = min(CH_W, WINDOWS - w)
                    state["shx_sb"] = spool.tile([P, CH_W, 4, 72], bf16, tag="shx", name="shx_sb")
                    nc.sync.dma_start(
                        state["shx_sb"][:, :cw, :, :].rearrange("p a b c -> p (a b c)"),
                        shx[:, g0 * 72 : (g0 + 4 * cw) * 72],
                    )
                    state["ohs_sb"] = spool.tile([P, CH_W, 4, 128], bf16, tag="ohs", name="ohs_sb")
                    nc.sync.dma_start(
                        state["ohs_sb"][:, :cw, :, :].rearrange("p a b c -> p (a b c)"),
                        ohs[:, g0 * 128 : (g0 + 4 * cw) * 128],
                    )

                if w % 2 == 0:
                    issue_gather(w // 2)

                # MLP1: h[comp, edge] for 512 edges, two 128-comp halves
                h_ps = pp1.tile([P, 2, 512], f32, space="PSUM", tag="hps")
                for half in range(2):
                    nc.tensor.matmul(
                        h_ps[:, half, :],
                        lhsT=w1_sb[32 * c : 32 * c + 8, half * 128 : half * 128 + 128],
                        rhs=es_sb[32 * c : 32 * c + 8, jj * 512 : jj * 512 + 512],
                        start=True, stop=True,
                        tile_position=(32 * c, 0),
                    )
                h_sb = wpool.tile([P, 2, 512], bf16, tag="hsb")
                nc.scalar.activation(out=h_sb[:], in_=h_ps[:], func=RELU)

                # MLP2 per group: w[edge, 256] in PSUM -> bf16 SBUF copy
                w_ps = pp.tile([P, 4, 256], f32, space="PSUM", tag="wps")
                for gg in range(4):
                    for half in range(2):
                        nc.tensor.matmul(
                            w_ps[:, gg, :],
                            lhsT=h_sb[:, half, gg * 128 : gg * 128 + 128],
                            rhs=w2_sb[:, half, :],
                            start=(half == 0), stop=(half == 1),
                        )
                w_sb = wpool.tile([P, 4, 256], bf16, tag="wsb")
                nc.scalar.copy(w_sb[:], w_ps[:])

                ctx[w] = (xcs[w // 2], state["shx_sb"], state["ohs_sb"], w_sb)

            def consume(w):
                g0 = 4 * w
                lw = w % CH_W
                x_c, shx_sb, ohs_sb, w_sb = ctx.pop(w)
                if w % 2 == 1:
                    xcs.pop(w // 2, None)
                xs = x_c[:, 4 * (w % 2) : 4 * (w % 2) + 4, :]   # [P, 4, 128]

                # TP products (batched over the 4 groups, all bf16)
                shw = shx_sb[:, lw, :, :]             # [P, 4, 72]
                s2r = shw[:, :, 0:24]                 # s2 repeated 24x
                v2A = shw[:, :, 24:48]                # v2 as (u8, i3)
                v2B = shw[:, :, 48:72]                # v2 as (i3, u8)

                s1s2 = wpool.tile([P, 4, 8], bf16, tag="s1s2")
                nc.vector.tensor_tensor(
                    out=s1s2[:], in0=xs[:, :, 0:8], in1=s2r[:, :, 0:8], op=MUL_
                )
                pb = wpool.tile([P, 4, 24], bf16, tag="pb")
                nc.vector.tensor_tensor(
                    out=pb[:], in0=xs[:, :, 8:32], in1=v2A, op=MUL_
                )
                bb = wpool.tile([P, 4, 8], bf16, tag="bb")
                with nc.allow_low_precision("bf16 dot over i=3"):
                    nc.vector.tensor_reduce(
                        out=bb[:],
                        in_=pb[:].rearrange("p g (u i) -> p g u i", u=8),
                        axis=AX, op=ADD,
                    )
                s1v2 = wpool.tile([P, 4, 24], bf16, tag="s1v2")
                nc.vector.tensor_tensor(
                    out=s1v2[:].rearrange("p g (i u) -> p g i u", i=3),
                    in0=xs[:, :, 0:8].unsqueeze(2).to_broadcast([P, 4, 3, 8]),
                    in1=v2B.rearrange("p g (i u) -> p g i u", i=3),
                    op=MUL_,
                )
                v1s2 = wpool.tile([P, 4, 24], bf16, tag="v1s2")
                nc.vector.tensor_tensor(
                    out=v1s2[:].rearrange("p g (i u) -> p g i u", i=3),
                    in0=xs[:, :, 8:32].rearrange("p g (u i) -> p g i u", u=8),
                    in1=s2r[:].rearrange("p g (i u) -> p g i u", i=3),
                    op=MUL_,
                )

                scat = wpool.tile([P, 4, 512], bf16, tag="scat")
                # path01: scat[(w8, u'16)] = [s1s2 | bb] * w01(w, u')
                w01 = w_sb[:, :, 0:128].rearrange("p g (w u) -> p g w u", w=8)
                sc01 = scat[:, :, 0:128].rearrange("p g (w u) -> p g w u", w=8)
                nc.vector.tensor_tensor(
                    out=sc01[:, :, :, 0:8],
                    in0=s1s2[:].unsqueeze(2).to_broadcast([P, 4, 8, 8]),
                    in1=w01[:, :, :, 0:8], op=MUL_,
                )
                nc.vector.tensor_tensor(
                    out=sc01[:, :, :, 8:16],
                    in0=bb[:].unsqueeze(2).to_broadcast([P, 4, 8, 8]),
                    in1=w01[:, :, :, 8:16], op=MUL_,
                )
                # path2: scat[(i3, w8, u8)] = s1v2(i,u) * w2(w,u)
                nc.vector.tensor_tensor(
                    out=scat[:, :, 128:320].rearrange(
                        "p g (i w u) -> p g i w u", i=3, w=8
                    ),
                    in0=s1v2[:].rearrange("p g (i u) -> p g i u", i=3)
                        .unsqueeze(3).to_broadcast([P, 4, 3, 8, 8]),
                    in1=w_sb[:, :, 128:192]
                        .rearrange("p g (w u) -> p g w u", w=8)
                        .unsqueeze(2).to_broadcast([P, 4, 3, 8, 8]),
                    op=MUL_,
                )
                # path3: scat[(i3, w8, u8)] = v1s2(i,u) * w3(w,u)
                nc.vector.tensor_tensor(
                    out=scat[:, :, 320:512].rearrange(
                        "p g (i w u) -> p g i w u", i=3, w=8
                    ),
                    in0=v1s2[:].rearrange("p g (i u) -> p g i u", i=3)
                        .unsqueeze(3).to_broadcast([P, 4, 3, 8, 8]),
                    in1=w_sb[:, :, 192:256]
                        .rearrange("p g (w u) -> p g w u", w=8)
                        .unsqueeze(2).to_broadcast([P, 4, 3, 8, 8]),
                    op=MUL_,
                )

                # per group: streamed one-hot + scatter matmul into block acc
                for gg in range(4):
                    g = g0 + gg
                    b, first, last = layout[g]
                    if first:
                        state["acc_ps"] = pp.tile(
                            [P, 512], f32, space="PSUM", tag="acc", name="acc_ps"
                        )
                    acc_ps = state["acc_ps"]
                    nc.tensor.matmul(
                        acc_ps[:],
                        lhsT=ohs_sb[:, lw, gg, :], rhs=scat[:, gg, :],
                        start=first, stop=last,
                    )
                    if last:
                        stage = wpool.tile([P, 32], f32, tag="stage")
                        nc.vector.tensor_reduce(
                            out=stage[:, 0:8],
                            in_=acc_ps[:, 0:128].rearrange(
                                "p (w u) -> p w u", w=8
                            ),
                            axis=AX, op=ADD,
                        )
                        nc.vector.tensor_reduce(
                            out=stage[:, 8:32],
                            in_=acc_ps[:, 128:512].rearrange(
                                "p (t i w u) -> p w i t u", t=2, i=3, w=8
                            ),
                            axis=AXY, op=ADD,
                        )
                        nc.sync.dma_start(
                            nodeout[128 * b : 128 * b + 128, :], stage[:]
                        )

            for w in range(WINDOWS + 1):
                if w < WINDOWS:
                    produce(w)
                if w >= 1:
                    consume(w - 1)
    nc.compile()
    return nc


# ---------------- host-side prep ----------------
def _prep(node_features, edge_src, edge_dst, edge_sh, edge_scalars, fc_w1, fc_w2,
          ngb, assign):
    ngb = np.asarray(ngb, np.int64)
    GROUPS = int(ngb.sum())
    EPAD = GROUPS * 128
    WINDOWS = GROUPS // 4
    NSUPER = (WINDOWS + 1) // 2
    NJ = (WINDOWS + 3) // 4
    gstart = np.zeros(NB, np.int64)
    gstart[1:] = np.cumsum(ngb)[:-1]
    k_of_group = np.repeat(np.arange(NB), ngb)

    # fold all scalar coefficients into the weights
    w1s = (fc_w1 * (1.0 / math.sqrt(NUM_BASIS))).astype(np.float32)     # [8, 256]
    w2 = (fc_w2 * (SQRT2 / math.sqrt(HIDDEN))).astype(np.float64)       # [256, 256]
    w2 = w2.reshape(HIDDEN, 4, MUL, MUL)
    coef = np.array(
        [A_SCALAR, A_SCALAR * INV_SQRT3, A_VECTOR * INV_SQRT3, A_VECTOR * INV_SQRT3]
    ) * DEG_SCALE
    w2 = w2 * coef[None, :, None, None]
    # device col order: [p01 (w8,u'16) | p2 (w8,u8) | p3 (w8,u8)], value a[u,w]
    p01 = np.concatenate([w2[:, 0], w2[:, 1]], axis=1)      # [H, u'16, w8]
    w2dev = np.concatenate(
        [
            p01.transpose(0, 2, 1).reshape(HIDDEN, 128),
            w2[:, 2].transpose(0, 2, 1).reshape(HIDDEN, 64),
            w2[:, 3].transpose(0, 2, 1).reshape(HIDDEN, 64),
        ],
        axis=1,
    ).astype(np.float32)                                                # [256, 256]

    w1t = np.zeros((P, 256), BF16)
    for c in range(4):
        w1t[32 * c : 32 * c + 8] = w1s.astype(BF16)
    w2t = np.zeros((P, 512), BF16)
    w2t[:, 0:256] = w2dev[0:128].astype(BF16)
    w2t[:, 256:512] = w2dev[128:256].astype(BF16)

    tbl = np.zeros((TBL_ROWS, 128), BF16)
    tbl[1 : N_NODES + 1, 0:32] = node_features.astype(BF16)

    src_all = np.asarray(edge_src).astype(np.int64)
    dst_all = np.asarray(edge_dst).astype(np.int64)
    es_all = np.asarray(edge_scalars).astype(np.float32)
    sh_all = np.asarray(edge_sh).astype(np.float32)
    core_of = assign[dst_all >> 7]

    # per-super gather boundaries (1024 idx per super; last may be shorter)
    ends = [min((si + 1) * 1024, EPAD) - 1 for si in range(NSUPER)]
    end_set = set(ends)

    in_maps = []
    bops = []
    for cid in range(NCORES):
        myblocks = np.nonzero(assign == cid)[0]          # global block ids
        local_of = np.full(NBLK, -1, np.int64)
        local_of[myblocks] = np.arange(len(myblocks))
        sel = np.nonzero(core_of == cid)[0]
        d = dst_all[sel]
        blk = local_of[d >> 7]                           # local block id
        order = np.argsort(blk, kind="stable")
        sel = sel[order]
        d = d[order]
        blk = blk[order]
        cnt = np.bincount(blk, minlength=NB)
        block_of_pos = np.argsort(-cnt, kind="stable")   # position k -> local blk
        pos_of_block = np.empty(NB, np.int64)
        pos_of_block[block_of_pos] = np.arange(NB)
        assert np.all(cnt[block_of_pos] <= ngb * 128), cid
        # position k -> GLOBAL block id (or -1 for pad slots)
        gb_of_pos = np.full(NB, -1, np.int64)
        valid = block_of_pos < len(myblocks)
        gb_of_pos[valid] = myblocks[block_of_pos[valid]]
        bops.append(gb_of_pos)
        start = np.zeros(NB, np.int64)
        start[1:] = np.cumsum(cnt)[:-1]
        rank = np.arange(len(sel)) - start[blk]
        slot = gstart[pos_of_block[blk]] * 128 + rank

        srcv = np.full(EPAD, -1, np.int64)
        srcv[slot] = src_all[sel]
        shv = np.zeros((EPAD, 4), np.float32)
        shv[slot] = sh_all[sel]
        esv = np.zeros((EPAD, 8), np.float32)
        esv[slot] = es_all[sel]
        dlv = np.full(EPAD, -1.0, np.float32)
        dlv[slot] = (d & 127).astype(np.float32)

        # gather indices: row = node+1, idx = row - GBASE; dummy -> DUMMY_IDX
        idxv = np.where(srcv >= 0, srcv + 1 - GBASE, DUMMY_IDX).astype(np.int64)
        # force the trim-order-last index of each gather call to be >= 0 by
        # swapping that edge with a non-negative-idx edge of the SAME node
        # block (any within-block permutation is valid).
        for jl in ends:
            if idxv[jl] >= 0:
                continue
            k0 = k_of_group[jl // 128]
            lo = int(gstart[k0]) * 128
            hi = lo + int(ngb[k0]) * 128
            cand = np.nonzero(idxv[lo:hi] >= 0)[0]
            cand = [lo + q for q in cand if (lo + q) not in end_set]
            assert cand, "no swap candidate in block"
            q = cand[0]
            for arr in (idxv, srcv, dlv):
                arr[jl], arr[q] = arr[q], arr[jl]
            for arr in (shv, esv):
                tmpq = arr[q].copy()
                arr[q] = arr[jl]
                arr[jl] = tmpq
        idx_g = np.tile(
            idxv.reshape(-1, 16).T.astype(np.int16), (8, 1)
        )  # [128, EPAD/16], wrap is uniform

        # es4: window w at rows 32*(w%4)+b, cols [ (w//4)*512, +512 )
        es4 = np.zeros((P, NJ * 512), BF16)
        esw = esv.reshape(WINDOWS, 512, 8)
        for c in range(4):
            wsel = np.arange(c, WINDOWS, 4)       # these windows use strip c
            nw = len(wsel)
            es4[32 * c : 32 * c + 8, : nw * 512] = (
                esw[wsel].transpose(2, 0, 1).reshape(8, nw * 512).astype(BF16)
            )

        # shx: [P, G, 72] = [s2 x24 | v2 as (u8,i3) | v2 as (i3,u8)]
        s2g = shv[:, 0].reshape(GROUPS, P).T                  # [P, G]
        v2g = shv[:, 1:4].reshape(GROUPS, P, 3).transpose(1, 0, 2)  # [P, G, 3]
        shx = np.empty((P, GROUPS, 72), BF16)
        shx[:, :, 0:24] = s2g[:, :, None]
        shx[:, :, 24:48] = np.tile(v2g, (1, 1, 8))
        shx[:, :, 48:72] = np.repeat(v2g, 8, axis=2)

        # ohs: [P, G, 128] one-hot of local dst (dummies: all-zero row)
        ohs = (
            dlv.reshape(GROUPS, P).T[:, :, None]
            == np.arange(128, dtype=np.float32)
        ).astype(BF16)

        in_maps.append(
            dict(
                tbl=tbl, idx_g=np.ascontiguousarray(idx_g),
                es4=np.ascontiguousarray(es4),
                shx=np.ascontiguousarray(shx.reshape(P, GROUPS * 72)),
                ohs=np.ascontiguousarray(ohs.reshape(P, GROUPS * 128)),
                w1t=w1t, w2t=w2t,
            )
        )
    return in_maps, bops


NBLK = (N_NODES + 127) // 128                       # 391 global 128-node blocks


def _compute_struct(edge_dst):
    """LPT-assign global blocks to cores (balance edge loads), then derive the
    shared group structure: ngb[k] = ceil(max over cores of the k-th largest
    per-core block count / 128)."""
    dst_all = np.asarray(edge_dst).astype(np.int64)
    cnt = np.bincount(dst_all >> 7, minlength=NBLK)
    order = np.argsort(-cnt, kind="stable")
    loads = np.zeros(NCORES, np.int64)
    nblocks = np.zeros(NCORES, np.int64)
    assign = np.zeros(NBLK, np.int64)
    for b in order:
        cands = [i for i in range(NCORES) if nblocks[i] < NB]
        c = min(cands, key=lambda i: loads[i])
        assign[b] = c
        loads[c] += cnt[b]
        nblocks[c] += 1
    percore = np.zeros((NCORES, NB), np.int64)
    for c in range(NCORES):
        sel = cnt[assign == c]
        percore[c, : len(sel)] = sel
    s = -np.sort(-percore, axis=1)                   # descending per core
    ngb = np.maximum(np.ceil(s.max(axis=0) / 128.0).astype(np.int64), 1)
    ngb[-1] += (-int(ngb.sum())) % 4                 # pad GROUPS to %4
    return tuple(int(v) for v in ngb), assign


def kernel(node_features, edge_src, edge_dst, edge_sh, edge_scalars, fc_w1, fc_w2):
    node_features = np.asarray(node_features, dtype=np.float32)
    edge_sh = np.asarray(edge_sh, dtype=np.float32)
    edge_scalars = np.asarray(edge_scalars, dtype=np.float32)
    fc_w1 = np.asarray(fc_w1, dtype=np.float32)
    fc_w2 = np.asarray(fc_w2, dtype=np.float32)

    ngb, assign = _compute_struct(edge_dst)
    if ngb not in _PROG_CACHE:
        _PROG_CACHE[ngb] = _build_program(ngb)
    nc = _PROG_CACHE[ngb]

    in_maps, bops = _prep(
        node_features, edge_src, edge_dst, edge_sh, edge_scalars, fc_w1, fc_w2,
        ngb, assign,
    )
    res = run_bass_kernel_spmd(nc, in_maps, core_ids=list(range(NCORES)))
    return _assemble([res.results[c]["nodeout"] for c in range(NCORES)], bops)


def _assemble(nodeouts, bops):
    out = np.zeros((NBLK * 128, 32), np.float32)
    for c in range(NCORES):
        r = nodeouts[c]
        for k in range(NB):
            gb = bops[c][k]
            if gb >= 0:
                out[128 * gb : 128 * gb + 128] = r[128 * k : 128 * k + 128]
    return out[:N_NODES].astype(np.float32)
